# revision 4
# baseline (speedup 1.0000x reference)
"""GAT layer (N=16384, d=128) on 8 TRN2 NeuronCores — fully replicated
bucket tables (no collective).

Per the sharding hint, e_dst/e_src are replicated to every device (packed
host-side, O(N*d) prep like the baseline's transpose).  Each core builds
the full K=16-bucket suffix tables over ALL N rows (128 accumulating
matmuls) and runs the transposed epilogue on its own 2048 rows.  No
cross-core communication, so no collective rendezvous cost.
"""

import numpy as np

N, D, P = 16384, 128, 128
N_CORES = 8
ROWS = N // N_CORES  # 2048
T = ROWS // P        # 16 own blocks
TN = N // P          # 128 global blocks
NEG = 0.01

K = 16
K2 = 2 * K
LO, HI = -6.0, 6.0
DELTA = (HI - LO) / K

_built = {}


def _mk_ap(base, dims):
    from concourse.ap import AP

    return AP(base.tensor, base.offset, [list(d) for d in dims])


def _build_kernel():
    if "nc" in _built:
        return _built

    import concourse.bass as bass  # noqa: F401
    import concourse.mybir as mybir
    import concourse.tile as tile
    from concourse import bacc

    f32 = mybir.dt.float32
    bf16 = mybir.dt.bfloat16
    Act = mybir.ActivationFunctionType
    Alu = mybir.AluOpType

    nc = bacc.Bacc("TRN2", target_bir_lowering=False, debug=False,
                   num_devices=N_CORES)

    # full [h | 1] blocks for all N rows, j on partitions
    hpk_d = nc.dram_tensor("hpk", [P, TN * (D + 1)], bf16, kind="ExternalInput").ap()
    # e_dst per row, j on partitions, block on free; exp variants
    edc_d = nc.dram_tensor("edc", [P, TN], f32, kind="ExternalInput").ap()
    # +/- e_src of OWN rows, replicated on K2 partitions
    srw_d = nc.dram_tensor("srw", [K2, ROWS], f32, kind="ExternalInput").ap()
    edg_d = nc.dram_tensor("edg", [P, K], bf16, kind="ExternalInput").ap()
    csts_d = nc.dram_tensor("csts", [P, 8], f32, kind="ExternalInput").ap()
    aux_d = nc.dram_tensor("aux", [P, 2 * P], bf16, kind="ExternalInput").ap()
    out_d = nc.dram_tensor("outb", [P, T * D], bf16, kind="ExternalOutput").ap()

    G = 4

    with tile.TileContext(nc) as tc:
        with tc.tile_pool(name="singles", bufs=1) as singles:
            h_sb = singles.tile([P, TN, D + 1], bf16, tag="h_sb")
            edc = singles.tile([P, TN], f32, tag="edc")
            srw = singles.tile([K2, ROWS], f32, tag="srw")
            edg = singles.tile([P, K], bf16, tag="edg")
            csts = singles.tile([P, 8], f32, tag="csts")
            aux = singles.tile([P, 2 * P], bf16, tag="aux")

            F_c = singles.tile([P, TN], f32, tag="F_c")
            f_c = singles.tile([P, TN], f32, tag="f_c")
            cmp_all = singles.tile([P, TN, K], bf16, tag="cmp_all")
            st_all = singles.tile([P, TN, K2], bf16, tag="st_all")

            E_stk = singles.tile([K2, ROWS], bf16, tag="E_stk")
            sel = singles.tile([K2, ROWS], bf16, tag="sel")

            u_sb = singles.tile([K2, D + 1], bf16, tag="u_sb")
            uT_sb = singles.tile([P, K2], bf16, tag="uT_sb")
            tcw_sb = singles.tile([K2, D + 1], bf16, tag="tcw_sb")
            bwd = singles.tile([K2, D + 1], bf16, tag="bwd")

            rden = singles.tile([P, T], f32, tag="rden")
            htr = singles.tile([P, T, D], bf16, tag="htr")
            ex = singles.tile([P, T, D], bf16, tag="ex")
            tt = singles.tile([P, T, D], bf16, tag="tt")
            outf = singles.tile([P, T, D], bf16, tag="outf")

            # ---- input DMAs: h in 4 chunks across 2 queues ----
            nc.scalar.dma_start(out=edc, in_=edc_d)
            nc.scalar.dma_start(out=edg, in_=edg_d)
            nc.scalar.dma_start(out=srw, in_=srw_d)
            nc.scalar.dma_start(out=csts, in_=csts_d)
            nc.scalar.dma_start(out=aux, in_=aux_d)
            QC = TN // 4 * (D + 1)
            for q in range(4):
                eng = nc.sync if q % 2 == 0 else nc.scalar
                eng.dma_start(
                    out=h_sb[:, q * TN // 4 : (q + 1) * TN // 4, :],
                    in_=hpk_d[:, q * QC : (q + 1) * QC],
                )

            with (
                tc.tile_pool(name="psA", bufs=1, space="PSUM") as psA,
                tc.tile_pool(name="psB", bufs=1, space="PSUM") as psB,
            ):
                u_ps = psA.tile([K2, D + 1], f32, tag="u_ps")
                uT_ps = psA.tile([P, K2], bf16, tag="uT_ps")
                tcw_ps = psA.tile([K2, P], f32, tag="tcw_ps")
                box_ps = psB.tile([K2, D + 1], f32, tag="box_ps")

                nc.scalar.activation(F_c, edc, Act.Exp)
                nc.scalar.activation(f_c, edc, Act.Exp, scale=NEG)

                # ---- bucket step matrices over all N, in 4 sweeps ----
                edg_b4 = _mk_ap(edg[:], [edg[:].ap[0], [0, TN // 4], edg[:].ap[1]])
                for q in range(4):
                    qs = slice(q * TN // 4, (q + 1) * TN // 4)
                    edc_b = _mk_ap(
                        edc[:, qs], [edc[:].ap[0], [1, TN // 4], [0, K]]
                    )
                    F_b = _mk_ap(
                        F_c[:, qs], [F_c[:].ap[0], [1, TN // 4], [0, K]]
                    )
                    f_b = _mk_ap(
                        f_c[:, qs], [f_c[:].ap[0], [1, TN // 4], [0, K]]
                    )
                    nc.vector.tensor_tensor(
                        out=cmp_all[:, qs, :], in0=edg_b4, in1=edc_b, op=Alu.is_le
                    )
                    nc.vector.tensor_tensor(
                        out=st_all[:, qs, 0:K], in0=cmp_all[:, qs, :], in1=F_b,
                        op=Alu.mult,
                    )
                    nc.vector.tensor_tensor(
                        out=st_all[:, qs, K:K2], in0=cmp_all[:, qs, :], in1=f_b,
                        op=Alu.mult,
                    )

                # ---- full suffix tables over ALL N rows ----
                for t in range(TN):
                    nc.tensor.matmul(
                        u_ps, st_all[:, t, :], h_sb[:, t, :],
                        start=(t == 0), stop=(t == TN - 1),
                    )

                # ---- selection matrix for own rows ----
                nc.scalar.activation(E_stk, srw, Act.Exp, scale=csts[0:K2, 1:2])
                nc.vector.scalar_tensor_tensor(
                    sel, srw, csts[0:K2, 0:1], E_stk,
                    op0=Alu.is_le, op1=Alu.mult,
                )

                # ---- project tables through W, box transform ----
                nc.scalar.copy(u_sb, u_ps)
                nc.tensor.transpose(uT_ps, u_sb[:, 0:P], aux[0:K2, 0:K2])
                nc.scalar.copy(uT_sb, uT_ps)
                nc.tensor.matmul(tcw_ps, uT_sb, aux[:, P : 2 * P],
                                 start=True, stop=True)
                nc.scalar.copy(tcw_sb[:, 0:P], tcw_ps)
                nc.scalar.copy(tcw_sb[:, P : P + 1], u_ps[:, P : P + 1])
                nc.tensor.matmul(box_ps, aux[0:K2, K2 : 2 * K2], tcw_sb,
                                 start=True, stop=True)
                nc.scalar.copy(bwd, box_ps)

            with tc.tile_pool(name="psE", bufs=1, space="PSUM") as psE:
                BK = 512
                po0 = psE.tile([P, G, BK], f32, tag="po0")
                po1 = psE.tile([P, G, BK], f32, tag="po1")
                for g in range(T // G):
                    po = po0 if g % 2 == 0 else po1
                    for k in range(G):
                        t = g * G + k
                        nc.tensor.matmul(
                            po[:, k, 0 : D + 1],
                            sel[:, t * P : (t + 1) * P], bwd,
                            start=True, stop=True,
                        )
                    gsl = slice(g * G, (g + 1) * G)
                    den_cols = _mk_ap(
                        po[:, 0, D : D + 1], [po[:].ap[0], [BK, G]]
                    )
                    nc.vector.reciprocal_approx_fast(
                        out=rden[:, gsl], in_=den_cols
                    )
                    rden_b = _mk_ap(
                        rden[:, gsl], [rden[:].ap[0], [1, G], [0, D]]
                    )
                    nc.vector.tensor_tensor(
                        out=htr[:, gsl, :], in0=po[:, :, 0:D], in1=rden_b,
                        op=Alu.mult,
                    )
                    nc.scalar.activation(ex[:, gsl, :], htr[:, gsl, :], Act.Exp)
                    nc.vector.tensor_scalar(
                        out=tt[:, gsl, :], in0=ex[:, gsl, :],
                        scalar1=1.0, scalar2=-1.0, op0=Alu.min, op1=Alu.add,
                    )
                    nc.vector.tensor_tensor(
                        out=outf[:, gsl, :], in0=tt[:, gsl, :],
                        in1=htr[:, gsl, :], op=Alu.max,
                    )
                    nc.sync.dma_start(
                        out=out_d[:, g * G * D : (g + 1) * G * D],
                        in_=outf[:, gsl, :],
                    )

    nc.compile()
    _built["nc"] = nc
    return _built


def kernel(h, W, a_src, a_dst, _trace=False, _trace_kwargs=None):
    import ml_dtypes
    from concourse.bass_utils import run_bass_kernel_spmd

    h = np.asarray(h, dtype=np.float32)
    W = np.asarray(W, dtype=np.float32)
    a_src = np.asarray(a_src, dtype=np.float32)
    a_dst = np.asarray(a_dst, dtype=np.float32)

    built = _build_kernel()
    nc = built["nc"]

    bf = ml_dtypes.bfloat16
    w_src = W @ a_src
    w_dst = W @ a_dst

    h_bf = h.astype(bf)
    # replicated e_dst / e_src (the sharding hint's "replicated e_dst")
    e_dst = (h_bf.astype(np.float32) @ w_dst.astype(bf).astype(np.float32))
    e_src = (h_bf.astype(np.float32) @ w_src.astype(bf).astype(np.float32))

    edc = np.ascontiguousarray(
        e_dst.reshape(TN, P).T.astype(np.float32)
    )  # [128 j, 128 t]

    edges = (LO + np.arange(K) * DELTA).astype(np.float32)
    edg = np.tile(edges[None, :], (P, 1)).astype(bf)

    centers = edges + DELTA / 2
    csts = np.zeros((P, 8), dtype=np.float32)
    csts[0:K, 0] = centers
    csts[K:K2, 0] = -centers
    csts[0:K, 1] = -1.0
    csts[K:K2, 1] = NEG

    bidiag = np.zeros((K, K), dtype=np.float32)
    bidiag[np.arange(K), np.arange(K)] = 1.0
    bidiag[np.arange(1, K), np.arange(K - 1)] = -1.0
    bd2 = np.zeros((K2, K2), dtype=np.float32)
    bd2[0:K, 0:K] = bidiag
    bd2[K:K2, K:K2] = bidiag

    aux = np.zeros((P, 2 * P), dtype=np.float32)
    aux[0:K2, 0:K2] = np.eye(K2)        # transpose identity
    aux[0:K2, K2 : 2 * K2] = bd2        # box transform
    aux[:, P : 2 * P] = W
    aux = aux.astype(bf)

    hpk = np.empty((P, TN * (D + 1)), dtype=bf)
    blk = hpk.reshape(P, TN, D + 1)
    blk[:, :, 0:D] = h_bf.reshape(TN, P, D).transpose(1, 0, 2)
    blk[:, :, D] = np.float32(1.0)

    in_maps = []
    for c in range(N_CORES):
        r0 = c * ROWS
        es = e_src[r0 : r0 + ROWS]
        srw = np.empty((K2, ROWS), dtype=np.float32)
        srw[0:K] = -es[None, :]
        srw[K:K2] = +es[None, :]
        in_maps.append(
            {
                "hpk": hpk,
                "edc": edc,
                "srw": srw,
                "edg": edg,
                "csts": csts,
                "aux": aux,
            }
        )

    res = run_bass_kernel_spmd(
        nc,
        in_maps,
        core_ids=list(range(N_CORES)),
        trace=_trace,
        **(_trace_kwargs or {}),
    )
    _built["last_result"] = res

    out = np.empty((N, D), dtype=np.float32)
    for c in range(N_CORES):
        ob = res.results[c]["outb"].reshape(P, T, D).astype(np.float32)
        out[c * ROWS : (c + 1) * ROWS] = ob.transpose(1, 0, 2).reshape(ROWS, D)
    return out
